# revision 1
# baseline (speedup 1.0000x reference)
"""Per-sample depthwise 7x7 SAME cross-correlation on 8 trn2 NeuronCores.

Problem: inputs [32,128,128,128] (B,H,W,C), kernels [32,7,7,128] (B,KH,KW,C).
out[b,y,x,c] = sum_{i,j} inputs[b, y+i-3, x+j-3, c] * kernels[b,i,j,c]

Strategy (pure data parallel, batch sharded 4 samples/core):
  - Host: transpose to channel-major [b, c, y, x], zero-pad spatially to
    134x134 so every tap is a plain shifted AP read (SAME padding built in).
  - On-chip layout: C=128 on partitions, (y, x) in the free dim. The
    per-(b,c) kernel tap value is a per-partition scalar, so each tap is one
    fused multiply-accumulate: scalar_tensor_tensor(acc = x_shift * w + acc).
  - Taps are split across VectorE (fused MACs, 32 taps) and GpSimdE (adds of
    per-partition-scaled products that ScalarE produces, 17 taps), so all
    three elementwise-capable engines run concurrently; the two partial
    accumulators are merged on VectorE and DMA'd out channel-major.
  - Host transposes the gathered result back to [B,H,W,C].

Why not the TensorEngine: a depthwise conv with per-(b,c) kernels has no
shared contraction — any matmul formulation either needs per-channel banded
weight matrices (whose on-chip materialization costs more than the conv
itself: 3584 128x128 bands vs 512 images) or wastes >=127/128 of the array
on diagonal weights. The elementwise path on VectorE is the real roofline.
"""

import numpy as np

import concourse.bass as bass
import concourse.tile as tile
from concourse import bacc, mybir
from concourse.bass_utils import run_bass_kernel_spmd

B, H, W, C = 32, 128, 128, 128
KH = KW = 7
PAD = 3
N_CORES = 8
BPC = B // N_CORES  # samples per core
HP, WP = H + 2 * PAD, W + 2 * PAD  # 134, 134
SLAB = 32  # output rows per compute slab
N_SLABS = H // SLAB

# Tap split across the engines (tuned via cost-model + HW sweep).
_ALL_TAPS = [(i, j) for i in range(KH) for j in range(KW)]
N_GP_TAPS = 18
_GP_TAPS = _ALL_TAPS[:N_GP_TAPS]
_DVE_TAPS = _ALL_TAPS[N_GP_TAPS:]
# Independent VectorE accumulator chains: back-to-back dependent DVE ops pay
# a pipeline DRAIN ~= op duration (measured 2.15x); interleaved independent
# chains overlap it (measured 1.88x recovery on a DVE-only variant).
N_DVE_CHAINS = 3

_PROGRAM_CACHE = {}


def _build_program(repeat=1):
    f32 = mybir.dt.float32
    nc = bacc.Bacc("TRN2", target_bir_lowering=False, debug=False)
    x_h = nc.dram_tensor("x", [BPC, C, HP, WP], f32, kind="ExternalInput")
    w_h = nc.dram_tensor("w", [BPC, C, KH * KW], f32, kind="ExternalInput")
    o_h = nc.dram_tensor("o", [BPC, C, H, W], f32, kind="ExternalOutput")
    x, w, o = x_h.ap(), w_h.ap(), o_h.ap()

    with tile.TileContext(nc) as tc:
        with (
            tc.tile_pool(name="wpool", bufs=1) as wpool,
            tc.tile_pool(name="xpool", bufs=3) as xpool,
            tc.tile_pool(name="accd0", bufs=2) as accd0p,
            tc.tile_pool(name="accdx", bufs=1) as accdxp,
            tc.tile_pool(name="accg", bufs=2) as accgp,
            tc.tile_pool(name="accg1", bufs=1) as accg1p,
            tc.tile_pool(name="tmp", bufs=2) as tmpp,
        ):
            wall = wpool.tile([C, BPC, KH * KW], f32)
            for b in range(BPC):
                nc.sync.dma_start(out=wall[:, b, :], in_=w[b])

            for b, s in [
                (b, s)
                for _ in range(repeat)
                for b in range(BPC)
                for s in range(N_SLABS)
            ]:
                if True:
                    y0 = s * SLAB
                    xt = xpool.tile([C, SLAB + 2 * PAD, WP], f32)
                    nc.sync.dma_start(out=xt, in_=x[b, :, y0 : y0 + SLAB + 2 * PAD, :])

                    dacc = [
                        (accd0p if ch == 0 else accdxp).tile(
                            [C, SLAB, W], f32, name=f"dacc{ch}", tag=f"dacc{ch}"
                        )
                        for ch in range(N_DVE_CHAINS)
                    ]
                    if _GP_TAPS:
                        acc_g = accgp.tile([C, SLAB, W], f32)
                        acc_g1 = accg1p.tile([C, SLAB, W], f32)
                        gacc = [acc_g, acc_g1]
                    else:
                        acc_g = None

                    started = [False] * N_DVE_CHAINS
                    for t, (i, j) in enumerate(_DVE_TAPS):
                        ch = t % N_DVE_CHAINS
                        xin = xt[:, i : i + SLAB, j : j + W]
                        wsc = wall[:, b, i * KW + j : i * KW + j + 1]
                        if not started[ch]:
                            nc.vector.tensor_scalar_mul(dacc[ch], xin, wsc)
                            started[ch] = True
                        else:
                            nc.vector.scalar_tensor_tensor(
                                out=dacc[ch], in0=xin, scalar=wsc, in1=dacc[ch],
                                op0=mybir.AluOpType.mult, op1=mybir.AluOpType.add,
                            )
                    # GpSimd side: 2 interleaved accumulator chains (same
                    # drain-overlap rationale as the VectorE chains); ScalarE
                    # seeds each chain and produces every product.
                    gstarted = [False, False]
                    for t, (i, j) in enumerate(_GP_TAPS):
                        gch = t % 2
                        xin = xt[:, i : i + SLAB, j : j + W]
                        wsc = wall[:, b, i * KW + j : i * KW + j + 1]
                        if not gstarted[gch]:
                            nc.scalar.mul(gacc[gch], xin, wsc)
                            gstarted[gch] = True
                        else:
                            prod = tmpp.tile([C, SLAB, W], f32)
                            nc.scalar.mul(prod, xin, wsc)
                            nc.gpsimd.tensor_add(gacc[gch], gacc[gch], prod)
                    nc.gpsimd.tensor_add(acc_g, acc_g, acc_g1)
                    # Tree merge: first level is two INDEPENDENT adds whose
                    # pipeline drains overlap; only the final add is serial.
                    if N_DVE_CHAINS == 3 and acc_g is not None:
                        nc.vector.tensor_add(dacc[0], dacc[0], dacc[1])
                        nc.vector.tensor_add(dacc[2], dacc[2], acc_g)
                        nc.vector.tensor_add(dacc[0], dacc[0], dacc[2])
                    else:
                        for ch in range(1, N_DVE_CHAINS):
                            nc.vector.tensor_add(dacc[0], dacc[0], dacc[ch])
                        if acc_g is not None:
                            nc.vector.tensor_add(dacc[0], dacc[0], acc_g)
                    nc.sync.dma_start(out=o[b, :, y0 : y0 + SLAB, :], in_=dacc[0])

    nc.compile()
    return nc


def _get_program():
    if "nc" not in _PROGRAM_CACHE:
        _PROGRAM_CACHE["nc"] = _build_program()
    return _PROGRAM_CACHE["nc"]


def _prep_inputs(inputs, kernels):
    """Host-side shard + layout transform. Returns per-core input maps."""
    xt = _PROGRAM_CACHE.get("xt")
    if xt is None:
        xt = np.zeros((B, C, HP, WP), np.float32)
        _PROGRAM_CACHE["xt"] = xt
    xt[:, :, PAD : PAD + H, PAD : PAD + W] = np.transpose(inputs, (0, 3, 1, 2))
    wt = np.ascontiguousarray(
        np.transpose(kernels, (0, 3, 1, 2)).reshape(B, C, KH * KW)
    )
    in_maps = []
    for k in range(N_CORES):
        sl = slice(k * BPC, (k + 1) * BPC)
        in_maps.append({"x": xt[sl], "w": wt[sl]})
    return in_maps


def _gather_output(results):
    full = np.concatenate([r["o"] for r in results], axis=0)  # [B, C, H, W]
    return np.ascontiguousarray(np.transpose(full, (0, 2, 3, 1)))


def run_spmd(inputs, kernels, **spmd_kwargs):
    """Run on all 8 cores; returns (output, BassKernelResults)."""
    nc = _get_program()
    in_maps = _prep_inputs(np.asarray(inputs), np.asarray(kernels))
    res = run_bass_kernel_spmd(nc, in_maps, list(range(N_CORES)), **spmd_kwargs)
    return _gather_output(res.results), res


def kernel(inputs, kernels):
    out, _ = run_spmd(inputs, kernels)
    return out



# revision 8
# speedup vs baseline: 3.2986x; 3.2986x over previous
"""Per-sample depthwise 7x7 SAME cross-correlation on 8 trn2 NeuronCores.

Problem: inputs [32,128,128,128] (B,H,W,C), kernels [32,7,7,128] (B,KH,KW,C).
out[b,y,x,c] = sum_{i,j} inputs[b, y+i-3, x+j-3, c] * kernels[b,i,j,c]

Strategy (pure data parallel, batch sharded 4 samples/core):
  - Host: transpose to channel-major [b, c, y, x], zero-pad spatially to
    134x134 (SAME padding built in), convert the image to bf16.
  - On-chip layout: C=128 on partitions, (y, x) in the free dim. Each tap is
    a per-(b,c) scalar multiply of a shifted window, summed over 49 taps.
  - The 49 taps are split across FOUR compute lanes that run concurrently:

    * PE lane (N_PE taps): the TensorEngine multiplies a whole shifted
      window by the per-channel tap scalar as a matmul with a DIAGONAL
      stationary matrix diag(w[b,:,tap]), accumulating all its taps into
      PSUM in f32 for free. The diagonal stationary is built in one DVE
      tensor_scalar op per (sample, tap): IDENT * w (IDENT is a constant
      0/1 identity tile). bf16 moving data streams 1 column/cycle at
      2.4 GHz -> 0.417 ns/elem/tap, ~2.4x any elementwise engine.
    * Act lane: ScalarE computes bf16 products (0.83 ns/elem,
      dtype-agnostic); DVE accumulates them at the bf16 2x rate (0.52).
    * DVE lane: self-contained products at the bf16 4x tensor_scalar rate
      (0.26) + 2x adds (0.52).
    * Pool lane: GpSimd accumulates Act-produced products with its Add
      ucode (1.98 ns/elem); f32 accumulator. (Pool supports no
      tensor_scalar/stt opcodes on TRN2, so it cannot make products.)

  - PSUM: each 16-row slab uses 4 single-bank [128,4,128] f32 tiles
    (matmul moving free dim is capped at 512), double-buffered = 8 banks.
  - Merge on DVE: A0 += A1 (bf16 2x), A0 += P (mixed), then four
    quarter-adds OUT[q] = A0[q] + PSUM[q] producing the f32 output.
  - Accuracy: PE/Pool lanes accumulate in f32; only ~13 taps ride bf16
    accumulators -> measured ~3e-3 max rel err vs f32 reference
    (harness gate 2e-2).
"""

import numpy as np
import ml_dtypes

import concourse.bass as bass
import concourse.tile as tile
from concourse import bacc, mybir
from concourse.bass_utils import run_bass_kernel_spmd

B, H, W, C = 32, 128, 128, 128
KH = KW = 7
PAD = 3
N_CORES = 8
BPC = B // N_CORES  # samples per core
HP, WP = H + 2 * PAD, W + 2 * PAD  # 134, 134
SLAB = 16  # output rows per compute slab (fits 4 psum banks)
N_SLABS = H // SLAB
QROWS = 4  # rows per matmul: 4*128 = 512 = max moving free size

# Tap split across the lanes (balanced via the instruction cost model).
N_PE = 29    # diag-matmul taps on the TensorEngine
N_ACT = 6    # ScalarE products accumulated by DVE
N_DVE = 7    # product+add pairs fully on DVE
N_POOL = 49 - N_PE - N_ACT - N_DVE  # Act products accumulated by GpSimd
POOL_CHAINS = 1

_PROGRAM_CACHE = {}


def _build_program(repeat=1, n_pe=N_PE, n_act=N_ACT, n_dve=N_DVE, slab=SLAB,
                   pool_chains=POOL_CHAINS):
    f32 = mybir.dt.float32
    bf16 = mybir.dt.bfloat16
    n_pool = 49 - n_pe - n_act - n_dve
    assert n_act >= 2 and n_pool >= pool_chains and n_pe >= 1
    taps = [(i, j) for i in range(KH) for j in range(KW)]
    pe_taps = taps[:n_pe]
    a_taps = taps[n_pe : n_pe + n_act]
    d_taps = taps[n_pe + n_act : n_pe + n_act + n_dve]
    g_taps = taps[n_pe + n_act + n_dve :]
    n_slabs = H // slab
    nq = slab // QROWS
    mult, add = mybir.AluOpType.mult, mybir.AluOpType.add

    nc = bacc.Bacc("TRN2", target_bir_lowering=False, debug=False)
    x_h = nc.dram_tensor("x", [BPC, C, HP, WP], bf16, kind="ExternalInput")
    w_h = nc.dram_tensor("w", [BPC, C, KH * KW], f32, kind="ExternalInput")
    o_h = nc.dram_tensor("o", [BPC, C, H, W], f32, kind="ExternalOutput")
    x, w, o = x_h.ap(), w_h.ap(), o_h.ap()

    with tile.TileContext(nc) as tc:
        with (
            tc.tile_pool(name="wpool", bufs=1) as wpool,
            tc.tile_pool(name="dpool", bufs=2) as dpool,
            tc.tile_pool(name="xpool", bufs=3) as xpool,
            tc.tile_pool(name="a0p", bufs=2) as a0p,
            tc.tile_pool(name="a1p", bufs=2) as a1p,
            tc.tile_pool(name="pcp", bufs=2) as pcp,
            tc.tile_pool(name="tmpa", bufs=4) as tmpap,
            tc.tile_pool(name="tmpd", bufs=2) as tmpdp,
            tc.tile_pool(name="outp", bufs=2) as outp,
            tc.psum_pool(name="ps", bufs=2) as ps,
        ):
            ones = wpool.tile([C, 128], bf16)
            ident = wpool.tile([C, 128], bf16)
            nc.vector.memset(ones, 1.0)
            nc.gpsimd.affine_select(
                out=ident, in_=ones, pattern=[[1, 128]],
                compare_op=mybir.AluOpType.is_equal, fill=0.0,
                base=0, channel_multiplier=-1,
            )
            wall = wpool.tile([C, BPC, KH * KW], f32)
            for b in range(BPC):
                nc.sync.dma_start(out=wall[:, b, :], in_=w[b])

            last_b = None
            for b, s in [
                (b, s)
                for _ in range(repeat)
                for b in range(BPC)
                for s in range(n_slabs)
            ]:
                if b != last_b:
                    # Per-sample diagonal stationaries for the PE taps.
                    diag = dpool.tile([C, n_pe, 128], bf16, name="diag")
                    for t, (i, j) in enumerate(pe_taps):
                        nc.vector.tensor_scalar_mul(
                            diag[:, t, :], ident,
                            wall[:, b, i * KW + j : i * KW + j + 1],
                        )
                    last_b = b

                y0 = s * slab
                xt = xpool.tile([C, slab + 2 * PAD, WP], bf16)
                nc.sync.dma_start(out=xt, in_=x[b, :, y0 : y0 + slab + 2 * PAD, :])

                def xin(t, r0=0, r1=slab):
                    i, j = t
                    return xt[:, i + r0 : i + r1, j : j + W]

                def wsc(t):
                    i, j = t
                    return wall[:, b, t[0] * KW + t[1] : t[0] * KW + t[1] + 1]

                # --- PE lane: diag matmuls accumulating into PSUM --------
                pt = [
                    ps.tile([C, QROWS, W], f32, name=f"pt{q}", tag=f"pt{q}")
                    for q in range(nq)
                ]
                for t, tap in enumerate(pe_taps):
                    for q in range(nq):
                        nc.tensor.matmul(
                            out=pt[q][:, :, :],
                            lhsT=diag[:, t, :],
                            rhs=xin(tap, QROWS * q, QROWS * (q + 1)),
                            start=(t == 0),
                            stop=(t == len(pe_taps) - 1),
                        )

                # --- Act lane: bf16 products (2 seed the DVE accs) -------
                aacc = [
                    a0p.tile([C, slab, W], bf16, name="a0", tag="a0"),
                    a1p.tile([C, slab, W], bf16, name="a1", tag="a1"),
                ]
                nc.scalar.mul(aacc[0], xin(a_taps[0]), wsc(a_taps[0]))
                nc.scalar.mul(aacc[1], xin(a_taps[1]), wsc(a_taps[1]))
                # Pool accumulators seeded directly by Act (f32).
                pacc = [
                    pcp.tile([C, slab, W], f32, name=f"pc{k}", tag=f"pc{k}")
                    for k in range(pool_chains)
                ]
                for k in range(pool_chains):
                    nc.scalar.mul(pacc[k], xin(g_taps[k]), wsc(g_taps[k]))
                # Remaining products, interleaved so Pool is fed steadily.
                a_rest = [("a", t) for t in a_taps[2:]]
                g_rest = [("g", t) for t in g_taps[pool_chains:]]
                prods = []
                na, ng = len(a_rest), len(g_rest)
                ia = ig = 0
                for k in range(na + ng):
                    if ig < ng and (ia >= na or ig * (na + ng) <= k * ng):
                        prods.append(g_rest[ig]); ig += 1
                    else:
                        prods.append(a_rest[ia]); ia += 1
                act_out = []
                for kind, tap in prods:
                    tmp = tmpap.tile([C, slab, W], bf16, name="atmp")
                    nc.scalar.mul(tmp, xin(tap), wsc(tap))
                    act_out.append((kind, tmp))

                # --- Pool lane: accumulate its products ------------------
                gch = 0
                for kind, tmp in act_out:
                    if kind == "g":
                        nc.gpsimd.tensor_add(pacc[gch], pacc[gch], tmp)
                        gch = (gch + 1) % pool_chains

                # --- DVE lane: Act-product adds + own pairs --------------
                dve_stream = [("act", tmp) for kind, tmp in act_out
                              if kind == "a"]
                step = max(1, (len(dve_stream) + len(d_taps)) // max(1, len(d_taps)))
                for k, tap in enumerate(d_taps):
                    pos = min(len(dve_stream), (k + 1) * step - 1)
                    dve_stream.insert(pos, ("dve", tap))
                ch = 0
                for kind, payload in dve_stream:
                    if kind == "act":
                        nc.vector.tensor_add(aacc[ch], aacc[ch], payload)
                    else:
                        tmp = tmpdp.tile([C, slab, W], bf16, name="dtmp")
                        nc.vector.tensor_scalar_mul(tmp, xin(payload), wsc(payload))
                        nc.vector.tensor_add(aacc[ch], aacc[ch], tmp)
                    ch ^= 1

                # --- Merge + output --------------------------------------
                out_t = outp.tile([C, slab, W], f32, name="out_t")
                nc.vector.tensor_add(aacc[0], aacc[0], aacc[1])
                for k in range(1, pool_chains):
                    nc.vector.tensor_add(pacc[0], pacc[0], pacc[k])
                nc.vector.tensor_add(aacc[0], aacc[0], pacc[0])
                for q in range(nq):
                    nc.vector.tensor_add(
                        out_t[:, QROWS * q : QROWS * (q + 1), :],
                        aacc[0][:, QROWS * q : QROWS * (q + 1), :],
                        pt[q],
                    )
                nc.sync.dma_start(out=o[b, :, y0 : y0 + slab, :], in_=out_t)

    nc.compile()
    return nc


def _get_program():
    if "nc" not in _PROGRAM_CACHE:
        _PROGRAM_CACHE["nc"] = _build_program()
    return _PROGRAM_CACHE["nc"]


def _prep_inputs(inputs, kernels):
    """Host-side shard + layout transform. Returns per-core input maps."""
    xt = _PROGRAM_CACHE.get("xt")
    if xt is None:
        xt = np.zeros((B, C, HP, WP), ml_dtypes.bfloat16)
        _PROGRAM_CACHE["xt"] = xt
    xt[:, :, PAD : PAD + H, PAD : PAD + W] = np.transpose(
        inputs, (0, 3, 1, 2)
    ).astype(ml_dtypes.bfloat16)
    wt = np.ascontiguousarray(
        np.transpose(kernels, (0, 3, 1, 2)).reshape(B, C, KH * KW)
    )
    in_maps = []
    for k in range(N_CORES):
        sl = slice(k * BPC, (k + 1) * BPC)
        in_maps.append({"x": xt[sl], "w": wt[sl]})
    return in_maps


def _gather_output(results):
    full = np.concatenate([r["o"] for r in results], axis=0)  # [B, C, H, W]
    return np.ascontiguousarray(np.transpose(full, (0, 2, 3, 1)))


def run_spmd(inputs, kernels, **spmd_kwargs):
    """Run on all 8 cores; returns (output, BassKernelResults)."""
    nc = _get_program()
    in_maps = _prep_inputs(np.asarray(inputs), np.asarray(kernels))
    res = run_bass_kernel_spmd(nc, in_maps, list(range(N_CORES)), **spmd_kwargs)
    return _gather_output(res.results), res


def kernel(inputs, kernels):
    out, _ = run_spmd(inputs, kernels)
    return out


# revision 18
# speedup vs baseline: 3.5539x; 1.0774x over previous
"""Per-sample depthwise 7x7 SAME cross-correlation on 8 trn2 NeuronCores.

Problem: inputs [32,128,128,128] (B,H,W,C), kernels [32,7,7,128] (B,KH,KW,C).
out[b,y,x,c] = sum_{i,j} inputs[b, y+i-3, x+j-3, c] * kernels[b,i,j,c]

Strategy (pure data parallel, batch sharded 4 samples/core):
  - Host: transpose to channel-major [b, c, y, x], zero-pad spatially to
    134x134 (SAME padding built in), convert the image to bf16.
  - On-chip layout: C=128 on partitions, (y, x) in the free dim. Each tap is
    a per-(b,c) scalar multiply of a shifted window, summed over 49 taps.
  - The 49 taps are split across FOUR compute lanes that run concurrently:

    * PE lane (N_PE taps): the TensorEngine multiplies a whole shifted
      window by the per-channel tap scalar as a matmul with a DIAGONAL
      stationary matrix diag(w[b,:,tap]), accumulating all its taps into
      PSUM in f32 for free. The diagonal stationary is built in one DVE
      tensor_scalar op per (sample, tap): IDENT * w (IDENT is a constant
      0/1 identity tile). bf16 moving data streams 1 column/cycle at
      2.4 GHz -> 0.417 ns/elem/tap, ~2.4x any elementwise engine.
    * Act lane: ScalarE computes bf16 products (0.83 ns/elem,
      dtype-agnostic); DVE accumulates them at the bf16 2x rate (0.52).
    * DVE lane: self-contained products at the bf16 4x tensor_scalar rate
      (0.26) + 2x adds (0.52).
    * Pool lane: GpSimd accumulates Act-produced products with its Add
      ucode (1.98 ns/elem); f32 accumulator. (Pool supports no
      tensor_scalar/stt opcodes on TRN2, so it cannot make products.)

  - PSUM: each 16-row slab uses 4 single-bank [128,4,128] f32 tiles
    (matmul moving free dim is capped at 512), double-buffered = 8 banks.
  - Merge on DVE: A0 += A1 (bf16 2x), A0 += P (mixed), then four
    quarter-adds OUT[q] = A0[q] + PSUM[q] producing the f32 output.
  - Accuracy: PE/Pool lanes accumulate in f32; only ~13 taps ride bf16
    accumulators -> measured ~3e-3 max rel err vs f32 reference
    (harness gate 2e-2).
"""

import numpy as np
import ml_dtypes

import concourse.bass as bass
import concourse.tile as tile
from concourse import bacc, mybir
from concourse.bass_utils import run_bass_kernel_spmd

B, H, W, C = 32, 128, 128, 128
KH = KW = 7
PAD = 3
N_CORES = 8
BPC = B // N_CORES  # samples per core
HP, WP = H + 2 * PAD, W + 2 * PAD  # 134, 134
SLAB = 16  # output rows per compute slab (fits 4 psum banks)
N_SLABS = H // SLAB
QROWS = 4  # rows per matmul: 4*128 = 512 = max moving free size

# Tap split across the lanes (balanced via the instruction cost model).
N_PE = 29    # diag-matmul taps on the TensorEngine
N_ACT = 7    # ScalarE products accumulated by DVE
N_DVE = 8    # product+add pairs fully on DVE
N_POOL = 49 - N_PE - N_ACT - N_DVE  # Act products accumulated by GpSimd
POOL_CHAINS = 1

_PROGRAM_CACHE = {}


def _build_program(repeat=1, n_pe=N_PE, n_act=N_ACT, n_dve=N_DVE, slab=SLAB,
                   pool_chains=POOL_CHAINS, q_adds_on="dve", merge="dve",
                   evac="act"):
    f32 = mybir.dt.float32
    bf16 = mybir.dt.bfloat16
    n_pool = 49 - n_pe - n_act - n_dve
    assert n_act >= 2 and n_pool >= pool_chains and n_pe >= 1
    taps = [(i, j) for i in range(KH) for j in range(KW)]
    pe_taps = taps[:n_pe]
    a_taps = taps[n_pe : n_pe + n_act]
    d_taps = taps[n_pe + n_act : n_pe + n_act + n_dve]
    g_taps = taps[n_pe + n_act + n_dve :]
    n_slabs = H // slab
    nq = slab // QROWS
    mult, add = mybir.AluOpType.mult, mybir.AluOpType.add

    nc = bacc.Bacc("TRN2", target_bir_lowering=False, debug=False)
    x_h = nc.dram_tensor("x", [BPC, C, HP, WP], bf16, kind="ExternalInput")
    w_h = nc.dram_tensor("w", [BPC, C, KH * KW], f32, kind="ExternalInput")
    o_h = nc.dram_tensor("o", [BPC, C, H, W], f32, kind="ExternalOutput")
    x, w, o = x_h.ap(), w_h.ap(), o_h.ap()

    with tile.TileContext(nc) as tc:
        with (
            tc.tile_pool(name="wpool", bufs=1) as wpool,
            tc.tile_pool(name="dpool", bufs=2) as dpool,
            tc.tile_pool(name="xpool", bufs=3) as xpool,
            tc.tile_pool(name="a0p", bufs=2) as a0p,
            tc.tile_pool(name="a1p", bufs=2) as a1p,
            tc.tile_pool(name="pcp", bufs=2) as pcp,
            tc.tile_pool(name="tmpa", bufs=4) as tmpap,
            tc.tile_pool(name="tmpd", bufs=2) as tmpdp,
            tc.tile_pool(name="outp", bufs=2) as outp,
            tc.psum_pool(name="ps", bufs=2) as ps,
        ):
            ones = wpool.tile([C, 128], bf16)
            ident = wpool.tile([C, 128], bf16)
            nc.vector.memset(ones, 1.0)
            nc.gpsimd.affine_select(
                out=ident, in_=ones, pattern=[[1, 128]],
                compare_op=mybir.AluOpType.is_equal, fill=0.0,
                base=0, channel_multiplier=-1,
            )
            wall = wpool.tile([C, BPC, KH * KW], f32)
            for b in range(BPC):
                nc.sync.dma_start(out=wall[:, b, :], in_=w[b])

            def emit_finish(pending):
                """Deferred slab finish: PE folds accs into PSUM, evac, DMA."""
                pt, accs, fb, fy0 = pending
                for k, acc in enumerate(accs):
                    for q in range(nq):
                        nc.tensor.matmul(
                            out=pt[q][:, :, :],
                            lhsT=ident,
                            rhs=acc[:, QROWS * q : QROWS * (q + 1), :],
                            start=False,
                            stop=(k == len(accs) - 1),
                        )
                out_t = outp.tile([C, slab, W], f32, name="out_t")
                for q in range(nq):
                    dst = out_t[:, QROWS * q : QROWS * (q + 1), :]
                    if evac == "act":
                        nc.scalar.copy(dst, pt[q])
                    else:
                        nc.vector.tensor_copy(dst, pt[q])
                nc.sync.dma_start(out=o[fb, :, fy0 : fy0 + slab, :], in_=out_t)

            pending = None
            last_b = None
            for b, s in [
                (b, s)
                for _ in range(repeat)
                for b in range(BPC)
                for s in range(n_slabs)
            ]:
                if b != last_b:
                    # Per-sample diagonal stationaries for the PE taps.
                    diag = dpool.tile([C, n_pe, 128], bf16, name="diag")
                    for t, (i, j) in enumerate(pe_taps):
                        nc.vector.tensor_scalar_mul(
                            diag[:, t, :], ident,
                            wall[:, b, i * KW + j : i * KW + j + 1],
                        )
                    last_b = b

                y0 = s * slab
                xt = xpool.tile([C, slab + 2 * PAD, WP], bf16)
                nc.sync.dma_start(out=xt, in_=x[b, :, y0 : y0 + slab + 2 * PAD, :])

                def xin(t, r0=0, r1=slab):
                    i, j = t
                    return xt[:, i + r0 : i + r1, j : j + W]

                def wsc(t):
                    i, j = t
                    return wall[:, b, t[0] * KW + t[1] : t[0] * KW + t[1] + 1]

                # --- PE lane: diag matmuls accumulating into PSUM --------
                pt = [
                    ps.tile([C, QROWS, W], f32, name=f"pt{q}", tag=f"pt{q}")
                    for q in range(nq)
                ]
                for t, tap in enumerate(pe_taps):
                    for q in range(nq):
                        nc.tensor.matmul(
                            out=pt[q][:, :, :],
                            lhsT=diag[:, t, :],
                            rhs=xin(tap, QROWS * q, QROWS * (q + 1)),
                            start=(t == 0),
                            stop=(merge == "dve" and t == len(pe_taps) - 1),
                        )

                # --- Act lane: bf16 products (2 seed the DVE accs) -------
                aacc = [
                    a0p.tile([C, slab, W], bf16, name="a0", tag="a0"),
                    a1p.tile([C, slab, W], bf16, name="a1", tag="a1"),
                ]
                nc.scalar.mul(aacc[0], xin(a_taps[0]), wsc(a_taps[0]))
                nc.scalar.mul(aacc[1], xin(a_taps[1]), wsc(a_taps[1]))
                # Pool accumulators seeded directly by Act.
                pdt = f32 if merge == "dve" else bf16
                pacc = [
                    pcp.tile([C, slab, W], pdt, name=f"pc{k}", tag=f"pc{k}")
                    for k in range(pool_chains)
                ]
                for k in range(pool_chains):
                    nc.scalar.mul(pacc[k], xin(g_taps[k]), wsc(g_taps[k]))
                # Remaining products, interleaved so Pool is fed steadily.
                a_rest = [("a", t) for t in a_taps[2:]]
                g_rest = [("g", t) for t in g_taps[pool_chains:]]
                prods = []
                na, ng = len(a_rest), len(g_rest)
                ia = ig = 0
                for k in range(na + ng):
                    if ig < ng and (ia >= na or ig * (na + ng) <= k * ng):
                        prods.append(g_rest[ig]); ig += 1
                    else:
                        prods.append(a_rest[ia]); ia += 1
                act_out = []
                for kind, tap in prods:
                    tmp = tmpap.tile([C, slab, W], bf16, name="atmp")
                    nc.scalar.mul(tmp, xin(tap), wsc(tap))
                    act_out.append((kind, tmp))

                # --- Pool lane: accumulate its products ------------------
                gch = 0
                for kind, tmp in act_out:
                    if kind == "g":
                        nc.gpsimd.tensor_add(pacc[gch], pacc[gch], tmp)
                        gch = (gch + 1) % pool_chains

                # --- DVE lane: Act-product adds + own pairs --------------
                dve_stream = [("act", tmp) for kind, tmp in act_out
                              if kind == "a"]
                step = max(1, (len(dve_stream) + len(d_taps)) // max(1, len(d_taps)))
                for k, tap in enumerate(d_taps):
                    pos = min(len(dve_stream), (k + 1) * step - 1)
                    dve_stream.insert(pos, ("dve", tap))
                ch = 0
                for kind, payload in dve_stream:
                    if kind == "act":
                        nc.vector.tensor_add(aacc[ch], aacc[ch], payload)
                    else:
                        tmp = tmpdp.tile([C, slab, W], bf16, name="dtmp")
                        nc.vector.tensor_scalar_mul(tmp, xin(payload), wsc(payload))
                        nc.vector.tensor_add(aacc[ch], aacc[ch], tmp)
                    ch ^= 1

                # --- Merge + output --------------------------------------
                if merge == "dve":
                    out_t = outp.tile([C, slab, W], f32, name="out_t")
                    nc.vector.tensor_add(aacc[0], aacc[0], aacc[1])
                    for k in range(1, pool_chains):
                        nc.vector.tensor_add(pacc[0], pacc[0], pacc[k])
                    nc.vector.tensor_add(aacc[0], aacc[0], pacc[0])
                    q_eng = nc.vector if q_adds_on == "dve" else nc.gpsimd
                    for q in range(nq):
                        q_eng.tensor_add(
                            out_t[:, QROWS * q : QROWS * (q + 1), :],
                            aacc[0][:, QROWS * q : QROWS * (q + 1), :],
                            pt[q],
                        )
                    nc.sync.dma_start(out=o[b, :, y0 : y0 + slab, :], in_=out_t)
                else:
                    emit_finish((pt, aacc + pacc, b, y0))

    nc.compile()
    return nc


def _get_program():
    if "nc" not in _PROGRAM_CACHE:
        _PROGRAM_CACHE["nc"] = _build_program()
    return _PROGRAM_CACHE["nc"]


def _prep_inputs(inputs, kernels):
    """Host-side shard + layout transform. Returns per-core input maps."""
    xt = _PROGRAM_CACHE.get("xt")
    if xt is None:
        xt = np.zeros((B, C, HP, WP), ml_dtypes.bfloat16)
        _PROGRAM_CACHE["xt"] = xt
    xt[:, :, PAD : PAD + H, PAD : PAD + W] = np.transpose(
        inputs, (0, 3, 1, 2)
    ).astype(ml_dtypes.bfloat16)
    wt = np.ascontiguousarray(
        np.transpose(kernels, (0, 3, 1, 2)).reshape(B, C, KH * KW)
    )
    in_maps = []
    for k in range(N_CORES):
        sl = slice(k * BPC, (k + 1) * BPC)
        in_maps.append({"x": xt[sl], "w": wt[sl]})
    return in_maps


def _gather_output(results):
    full = np.concatenate([r["o"] for r in results], axis=0)  # [B, C, H, W]
    return np.ascontiguousarray(np.transpose(full, (0, 2, 3, 1)))


def run_spmd(inputs, kernels, **spmd_kwargs):
    """Run on all 8 cores; returns (output, BassKernelResults)."""
    nc = _get_program()
    in_maps = _prep_inputs(np.asarray(inputs), np.asarray(kernels))
    res = run_bass_kernel_spmd(nc, in_maps, list(range(N_CORES)), **spmd_kwargs)
    return _gather_output(res.results), res


def kernel(inputs, kernels):
    out, _ = run_spmd(inputs, kernels)
    return out
